# revision 1
# baseline (speedup 1.0000x reference)
"""BatchBlur_SV kernel for 8 Trainium2 NeuronCores (Bass/Tile).

Reference semantics (including its reshape-scrambling "bug"):
  X = ker.reshape(361, 65536)                  # (kernel-pos ab, pixel p)
  s1 = X.sum(0);  W  = X / s1                  # stage-1 per-pixel normalize
  A2 = W.flat chunks of 361; s2 = row sums;  B2 = A2 / s2     # stage 2
  A3 = (B2.T).flat chunks of 361; s3 = row sums               # stage 3
  U  = im2col(reflect_pad(input[0,2], 9)) in (ab, p) layout   # (361, 65536)
  out[r] = sum(U.flat_chunk_r * A3[r]) / s3[r]

All arithmetic runs on-device in 3 SPMD launches over 8 cores, each core
working on a 1/8 flat band. Host only slices / rolls / transposes between
launches (data movement, no math).
"""

import numpy as np

P = 65536          # pixels
L = 19
L2 = 361           # kernel positions
NCORES = 8
PS = P // NCORES   # 8192 rows per core
NB = PS * L2       # flat elements per band
G = 8              # subtiles per DMA group
NGRP = PS // (128 * G)   # 8 groups per core

_CACHE: dict = {}


def _f32():
    from concourse import mybir
    return mybir.dt.float32


def _grouped(ap):
    # (PS, L2) -> [g][k][(i j)] with row = g*1024 + k*G + i: each partition
    # holds G consecutive rows, so src/dst DMA patterns are contiguous 2D.
    return ap.rearrange("(g k i) j -> g k (i j)", g=NGRP, k=128, i=G)


def _build_k1():
    """colsum kernel: in xT (PS, 361) slice of X.T -> out s1 (PS,)"""
    import concourse.bacc as bacc
    import concourse.tile as tile
    from concourse import mybir

    nc = bacc.Bacc("TRN2", target_bir_lowering=False)
    xT = nc.dram_tensor("xT", [PS, L2], _f32(), kind="ExternalInput")
    s1 = nc.dram_tensor("s1", [128, NGRP * G], _f32(), kind="ExternalOutput")
    xr = _grouped(xT[:, :])
    s1r = s1[:, :]
    with tile.TileContext(nc) as tc:
        with (
            tc.tile_pool(name="io", bufs=3) as pool,
            tc.tile_pool(name="acc", bufs=1) as accp,
        ):
            acc = accp.tile([128, NGRP, G], _f32())
            for g in range(NGRP):
                xt = pool.tile([128, G, L2], _f32())
                nc.sync.dma_start(
                    out=xt[:, :, :].rearrange("k i j -> k (i j)"), in_=xr[g]
                )
                nc.vector.tensor_reduce(
                    out=acc[:, g, :], in_=xt,
                    axis=mybir.AxisListType.X, op=mybir.AluOpType.add,
                )
            nc.sync.dma_start(out=s1r, in_=acc)
    nc.compile()
    return nc


def _build_k2():
    """stage-2 kernel: in a2 (PS,361) = X.flat band, s1b (PS,361) = matching
    per-element stage-1 sums; out b2 (PS,361) normalized chunks."""
    import concourse.bacc as bacc
    import concourse.tile as tile
    from concourse import mybir

    nc = bacc.Bacc("TRN2", target_bir_lowering=False)
    a2 = nc.dram_tensor("a2", [PS, L2], _f32(), kind="ExternalInput")
    s1b = nc.dram_tensor("s1b", [PS, L2], _f32(), kind="ExternalInput")
    b2 = nc.dram_tensor("b2", [PS, L2], _f32(), kind="ExternalOutput")
    a2r, s1r, b2r = _grouped(a2[:, :]), _grouped(s1b[:, :]), _grouped(b2[:, :])
    with tile.TileContext(nc) as tc:
        with (
            tc.tile_pool(name="io", bufs=3) as pool,
            tc.tile_pool(name="w", bufs=3) as wpool,
            tc.tile_pool(name="st", bufs=3) as spool,
        ):
            for g in range(NGRP):
                ta = pool.tile([128, G, L2], _f32(), tag="ta")
                ts = pool.tile([128, G, L2], _f32(), tag="ts")
                nc.sync.dma_start(
                    out=ta[:, :, :].rearrange("k i j -> k (i j)"), in_=a2r[g]
                )
                nc.sync.dma_start(
                    out=ts[:, :, :].rearrange("k i j -> k (i j)"), in_=s1r[g]
                )
                tr = pool.tile([128, G, L2], _f32(), tag="tr")
                nc.vector.reciprocal(out=tr, in_=ts)
                tw = wpool.tile([128, G, L2], _f32())
                nc.vector.tensor_mul(out=tw, in0=ta, in1=tr)
                s2 = spool.tile([128, G], _f32(), tag="s2")
                nc.vector.tensor_reduce(
                    out=s2, in_=tw,
                    axis=mybir.AxisListType.X, op=mybir.AluOpType.add,
                )
                r2 = spool.tile([128, G], _f32(), tag="r2")
                nc.vector.reciprocal(out=r2, in_=s2)
                for i in range(G):
                    nc.vector.tensor_scalar_mul(
                        out=tw[:, i, :], in0=tw[:, i, :],
                        scalar1=r2[:, i : i + 1],
                    )
                nc.sync.dma_start(
                    out=b2r[g], in_=tw[:, :, :].rearrange("k i j -> k (i j)")
                )
    nc.compile()
    return nc


def _build_k3():
    """final kernel: in v (PS,361) = B2T.flat band, u (PS,361) = U.flat band;
    out o (PS,) = rowdot(u,v)/rowsum(v)."""
    import concourse.bacc as bacc
    import concourse.tile as tile
    from concourse import mybir

    nc = bacc.Bacc("TRN2", target_bir_lowering=False)
    v = nc.dram_tensor("v", [PS, L2], _f32(), kind="ExternalInput")
    u = nc.dram_tensor("u", [PS, L2], _f32(), kind="ExternalInput")
    o = nc.dram_tensor("o", [128, NGRP * G], _f32(), kind="ExternalOutput")
    vr, ur = _grouped(v[:, :]), _grouped(u[:, :])
    orr = o[:, :]
    with tile.TileContext(nc) as tc:
        with (
            tc.tile_pool(name="io", bufs=3) as pool,
            tc.tile_pool(name="pr", bufs=2) as prp,
            tc.tile_pool(name="st", bufs=3) as spool,
            tc.tile_pool(name="acc", bufs=1) as accp,
        ):
            oacc = accp.tile([128, NGRP, G], _f32())
            for g in range(NGRP):
                tv = pool.tile([128, G, L2], _f32(), tag="tv")
                tu = pool.tile([128, G, L2], _f32(), tag="tu")
                nc.sync.dma_start(
                    out=tv[:, :, :].rearrange("k i j -> k (i j)"), in_=vr[g]
                )
                nc.sync.dma_start(
                    out=tu[:, :, :].rearrange("k i j -> k (i j)"), in_=ur[g]
                )
                s3 = spool.tile([128, G], _f32(), tag="s3")
                nc.vector.tensor_reduce(
                    out=s3, in_=tv,
                    axis=mybir.AxisListType.X, op=mybir.AluOpType.add,
                )
                prod = prp.tile([128, G, L2], _f32())
                nc.vector.tensor_mul(out=prod, in0=tu, in1=tv)
                dots = spool.tile([128, G], _f32(), tag="dots")
                nc.vector.tensor_reduce(
                    out=dots, in_=prod,
                    axis=mybir.AxisListType.X, op=mybir.AluOpType.add,
                )
                r3 = spool.tile([128, G], _f32(), tag="r3")
                nc.vector.reciprocal(out=r3, in_=s3)
                nc.vector.tensor_mul(out=oacc[:, g, :], in0=dots, in1=r3)
            nc.sync.dma_start(out=orr, in_=oacc)
    nc.compile()
    return nc


def _run(key, builder, in_maps, trace=False):
    from concourse.bass_utils import run_bass_kernel_spmd

    if key not in _CACHE:
        _CACHE[key] = builder()
    res = run_bass_kernel_spmd(
        _CACHE[key], in_maps, core_ids=list(range(NCORES)), trace=trace
    )
    return res


def kernel(input, kernel):
    import os

    trace = bool(int(os.environ.get("BASSBLUR_TRACE", "0")))
    inp = np.ascontiguousarray(np.asarray(input, dtype=np.float32))
    ker = np.ascontiguousarray(np.asarray(kernel, dtype=np.float32))
    X = ker.reshape(L2, P)
    Xf = X.reshape(-1)

    times = []

    # ---- launch 1: s1 = column sums of X ------------------------------
    XT = X.T  # (P, 361) view
    in1 = [
        {"xT": np.ascontiguousarray(XT[m * PS : (m + 1) * PS])}
        for m in range(NCORES)
    ]
    r1 = _run("k1", _build_k1, in1, trace=trace)
    s1 = np.concatenate(
        [r["s1"].reshape(128, NGRP, G).transpose(1, 0, 2).ravel()
         for r in r1.results]
    )
    times.append(r1.exec_time_ns)

    # ---- launch 2: per-chunk stage-2 normalize ------------------------
    # band m covers flat [NB*m, NB*(m+1)); element x there needs
    # s1[(NB*m + x) % P]; NB % P == PS so the roll shift is PS*m.
    in2 = []
    for m in range(NCORES):
        s1b = np.resize(np.roll(s1, -(PS * m) % P), NB).reshape(PS, L2)
        in2.append(
            {
                "a2": Xf[NB * m : NB * (m + 1)].reshape(PS, L2),
                "s1b": np.ascontiguousarray(s1b),
            }
        )
    r2 = _run("k2", _build_k2, in2, trace=trace)
    B2 = np.concatenate([r["b2"] for r in r2.results], axis=0)  # (P, 361)
    times.append(r2.exec_time_ns)

    # ---- launch 3: final dot over B2T/U flat chunks -------------------
    B2Tf = np.ascontiguousarray(B2.T).reshape(-1)
    pad = np.pad(inp[0, 2], L // 2, mode="reflect")  # (274, 274)
    from numpy.lib.stride_tricks import sliding_window_view

    U = np.ascontiguousarray(
        sliding_window_view(pad, (256, 256)).reshape(L2, P)
    )
    Uf = U.reshape(-1)
    in3 = [
        {
            "v": B2Tf[NB * m : NB * (m + 1)].reshape(PS, L2),
            "u": Uf[NB * m : NB * (m + 1)].reshape(PS, L2),
        }
        for m in range(NCORES)
    ]
    r3 = _run("k3", _build_k3, in3, trace=trace)
    out = np.concatenate(
        [r["o"].reshape(128, NGRP, G).transpose(1, 0, 2).ravel()
         for r in r3.results]
    )
    times.append(r3.exec_time_ns)

    if trace:
        kernel._last_times_ns = times  # stash for test harness

    return out.reshape(1, 1, 256, 256).astype(np.float32)


def hw_time_estimate_ns():
    """Per-launch HW time from the instruction cost model (TimelineSim).

    NTFF/neuron-profile capture is unavailable under this axon build, so this
    is the principled substitute: the same InstructionCostModel the Tile
    scheduler uses, over the exact BIR that runs on the cores.
    """
    from concourse.timeline_sim import TimelineSim

    out = []
    for key, builder in [("k1", _build_k1), ("k2", _build_k2), ("k3", _build_k3)]:
        if key not in _CACHE:
            _CACHE[key] = builder()
        out.append(int(TimelineSim(_CACHE[key]).simulate()))
    return out



# revision 7
# speedup vs baseline: 1.4772x; 1.4772x over previous
"""BatchBlur_SV kernel for 8 Trainium2 NeuronCores (Bass/Tile).

Reference semantics (including its reshape-scrambling "bug"):
  X = ker.reshape(361, 65536)                  # (kernel-pos ab, pixel p)
  s1 = X.sum(0);  W  = X / s1                  # stage-1 per-pixel normalize
  A2 = W.flat chunks of 361; s2 = row sums;  B2 = A2 / s2     # stage 2
  A3 = (B2.T).flat chunks of 361; s3 = row sums               # stage 3
  U  = im2col(reflect_pad(input[0,2], 9)) in (ab, p) layout   # (361, 65536)
  out[r] = sum(U.flat_chunk_r * A3[r]) / s3[r]

All arithmetic runs on-device in 3 SPMD launches over 8 cores. Host only
slices / rolls / transposes / dtype-converts between launches (data
movement, no math).

Big streams travel as fp16 (values are bounded and the gate is rel_err
< 2e-2); accumulations are fp32.  Column-direction reductions (k1 colsum,
k3 dots/s3) run on the PE array as ones-vector matmuls over a transposed
band layout, which is nearly free next to DVE reductions.
"""

import numpy as np

P = 65536          # pixels
L = 19
L2 = 361           # kernel positions
L2P = 384          # L2 padded to 3*128 partitions
NCORES = 8
PS = P // NCORES   # 8192 rows per core
NB = PS * L2       # flat elements per band
G = 8              # subtiles per DMA group
NGRP = PS // (128 * G)   # 8 groups per core
NC = PS // 512     # 512-wide psum chunks per band (16)
CW = PS // 128     # per-partition width of [128, CW] strip relayout (64)

_CACHE: dict = {}


def _dt():
    from concourse import mybir
    return mybir.dt


def _grouped(ap):
    # (PS, L2) -> [g][k][(i j)] with row = g*1024 + k*G + i: each partition
    # holds G consecutive rows, so src/dst DMA patterns are contiguous 2D.
    return ap.rearrange("(g k i) j -> g k (i j)", g=NGRP, k=128, i=G)


def _build_k1():
    """colsum kernel: in xp (L2P, PS) = padded X[:, pband] transposed slab;
    out inv1 (128, CW) with inv1[k, c] = 1 / s1[pband_start + k*CW + c]."""
    import concourse.bacc as bacc
    import concourse.tile as tile
    from concourse import mybir

    dt = _dt()
    nc = bacc.Bacc("TRN2", target_bir_lowering=False)
    xp = nc.dram_tensor("xp", [L2P, PS], dt.float16, kind="ExternalInput")
    inv1 = nc.dram_tensor("inv1", [128, CW], dt.float32, kind="ExternalOutput")
    sc1 = nc.dram_tensor("sc1", [1, PS], dt.float32, kind="Internal")
    with tile.TileContext(nc) as tc:
        with (
            tc.tile_pool(name="io", bufs=1) as pool,
            tc.tile_pool(name="st", bufs=1) as spool,
            tc.psum_pool(name="ps", bufs=4) as psp,
        ):
            ones = spool.tile([128, 1], dt.float16)
            nc.vector.memset(ones, 1.0)
            xts = []
            for t in range(3):
                xt = pool.tile([128, PS], dt.float16, tag=f"x{t}")
                nc.sync.dma_start(out=xt, in_=xp[128 * t : 128 * (t + 1), :])
                xts.append(xt)
            strip = spool.tile([1, PS], dt.float32)
            for c in range(NC):
                ps = psp.tile([1, 512], dt.float32)
                for t in range(3):
                    nc.tensor.matmul(
                        ps[:, :], lhsT=ones,
                        rhs=xts[t][:, 512 * c : 512 * (c + 1)],
                        start=(t == 0), stop=(t == 2),
                    )
                eng = nc.vector.tensor_copy if c % 2 else nc.scalar.copy
                eng(out=strip[:, 512 * c : 512 * (c + 1)], in_=ps[:, :])
            nc.sync.dma_start(out=sc1[:, :], in_=strip)
            rt = spool.tile([128, CW], dt.float32)
            nc.sync.dma_start(
                out=rt, in_=sc1[:, :].rearrange("a (k c) -> (a k) c", k=128)
            )
            iv = spool.tile([128, CW], dt.float32)
            nc.vector.reciprocal(out=iv, in_=rt)
            nc.sync.dma_start(out=inv1[:, :], in_=iv)
    nc.compile()
    return nc


def _build_k2():
    """stage-2 kernel: in a2 (PS,361) = X.flat band, i1b (PS,361) = matching
    per-element stage-1 reciprocal sums; out b2 (PS,361) normalized chunks."""
    import concourse.bacc as bacc
    import concourse.tile as tile
    from concourse import mybir

    dt = _dt()
    nc = bacc.Bacc("TRN2", target_bir_lowering=False)
    a2 = nc.dram_tensor("a2", [PS, L2], dt.float16, kind="ExternalInput")
    i1b = nc.dram_tensor("i1b", [PS, L2], dt.float16, kind="ExternalInput")
    b2 = nc.dram_tensor("b2", [PS, L2], dt.float16, kind="ExternalOutput")
    a2r, i1r, b2r = _grouped(a2[:, :]), _grouped(i1b[:, :]), _grouped(b2[:, :])
    with tile.TileContext(nc) as tc:
        with (
            tc.tile_pool(name="io", bufs=3) as pool,
            tc.tile_pool(name="w", bufs=2) as wpool,
            tc.tile_pool(name="st", bufs=3) as spool,
        ):
            for g in range(NGRP):
                ta = pool.tile([128, G, L2], dt.float16, tag="ta")
                ti = pool.tile([128, G, L2], dt.float16, tag="ti")
                nc.sync.dma_start(
                    out=ta[:, :, :].rearrange("k i j -> k (i j)"), in_=a2r[g]
                )
                nc.sync.dma_start(
                    out=ti[:, :, :].rearrange("k i j -> k (i j)"), in_=i1r[g]
                )
                tw = wpool.tile([128, G, L2], dt.float16, tag="tw")
                nc.vector.tensor_mul(out=tw, in0=ta, in1=ti)
                s2 = spool.tile([128, G], dt.float32, tag="s2")
                nc.vector.tensor_reduce(
                    out=s2, in_=tw,
                    axis=mybir.AxisListType.X, op=mybir.AluOpType.add,
                )
                r2 = spool.tile([128, G], dt.float32, tag="r2")
                nc.vector.reciprocal(out=r2, in_=s2)
                tb = wpool.tile([128, G, L2], dt.float16, tag="tb")
                for i in range(G):
                    nc.vector.tensor_scalar_mul(
                        out=tb[:, i, :], in0=tw[:, i, :],
                        scalar1=r2[:, i : i + 1],
                    )
                nc.sync.dma_start(
                    out=b2r[g], in_=tb[:, :, :].rearrange("k i j -> k (i j)")
                )
    nc.compile()
    return nc


def _build_k3():
    """final kernel: in vT/uT (L2P, PS) = transposed+padded B2T/U flat bands;
    out o (128, CW) with o[k, c] = out[band_start + k*CW + c]."""
    import concourse.bacc as bacc
    import concourse.tile as tile
    from concourse import mybir

    dt = _dt()
    nc = bacc.Bacc("TRN2", target_bir_lowering=False)
    vT = nc.dram_tensor("vT", [L2P, PS], dt.float16, kind="ExternalInput")
    uT = nc.dram_tensor("uT", [L2P, PS], dt.float16, kind="ExternalInput")
    o = nc.dram_tensor("o", [128, CW], dt.float32, kind="ExternalOutput")
    scd = nc.dram_tensor("scd", [1, PS], dt.float16, kind="Internal")
    scs = nc.dram_tensor("scs", [1, PS], dt.float16, kind="Internal")
    with tile.TileContext(nc) as tc:
        with (
            tc.tile_pool(name="io", bufs=1) as pool,
            tc.tile_pool(name="pr", bufs=1) as prp,
            tc.tile_pool(name="st", bufs=1) as spool,
            tc.psum_pool(name="ps", bufs=4) as psp,
        ):
            ones = spool.tile([128, 1], dt.float16)
            nc.vector.memset(ones, 1.0)
            vts, uts, prods = [], [], []
            for t in range(3):
                vt = pool.tile([128, PS], dt.float16, tag=f"v{t}")
                ut = pool.tile([128, PS], dt.float16, tag=f"u{t}")
                nc.sync.dma_start(out=vt, in_=vT[128 * t : 128 * (t + 1), :])
                nc.sync.dma_start(out=ut, in_=uT[128 * t : 128 * (t + 1), :])
                pr = prp.tile([128, PS], dt.float16, tag=f"p{t}")
                nc.vector.tensor_mul(out=pr, in0=vt, in1=ut)
                vts.append(vt)
                uts.append(ut)
                prods.append(pr)
            sdot = spool.tile([1, PS], dt.float16, tag="sdot")
            ssum = spool.tile([1, PS], dt.float16, tag="ssum")
            for c in range(NC):
                sl = slice(512 * c, 512 * (c + 1))
                psd = psp.tile([1, 512], dt.float32, tag="psd")
                for t in range(3):
                    nc.tensor.matmul(
                        psd[:, :], lhsT=ones, rhs=prods[t][:, sl],
                        start=(t == 0), stop=(t == 2),
                    )
                pss = psp.tile([1, 512], dt.float32, tag="pss")
                for t in range(3):
                    nc.tensor.matmul(
                        pss[:, :], lhsT=ones, rhs=vts[t][:, sl],
                        start=(t == 0), stop=(t == 2),
                    )
                nc.vector.tensor_copy(out=sdot[:, sl], in_=psd[:, :])
                nc.scalar.copy(out=ssum[:, sl], in_=pss[:, :])
            nc.sync.dma_start(out=scd[:, :], in_=sdot)
            nc.sync.dma_start(out=scs[:, :], in_=ssum)
            td = spool.tile([128, CW], dt.float16, tag="td")
            ts = spool.tile([128, CW], dt.float16, tag="ts")
            nc.sync.dma_start(out=td, in_=scd[:, :].rearrange("a (k c) -> (a k) c", k=128))
            nc.sync.dma_start(out=ts, in_=scs[:, :].rearrange("a (k c) -> (a k) c", k=128))
            tr = spool.tile([128, CW], dt.float32, tag="tr")
            nc.vector.reciprocal(out=tr, in_=ts)
            to = spool.tile([128, CW], dt.float32, tag="to")
            nc.vector.tensor_mul(out=to, in0=td, in1=tr)
            nc.sync.dma_start(out=o[:, :], in_=to)
    nc.compile()
    return nc


def _run(key, builder, in_maps, trace=False):
    from concourse.bass_utils import run_bass_kernel_spmd

    if key not in _CACHE:
        _CACHE[key] = builder()
    res = run_bass_kernel_spmd(
        _CACHE[key], in_maps, core_ids=list(range(NCORES)), trace=trace
    )
    return res


def kernel(input, kernel):
    import os

    trace = bool(int(os.environ.get("BASSBLUR_TRACE", "0")))
    inp = np.ascontiguousarray(np.asarray(input, dtype=np.float32))
    ker = np.ascontiguousarray(np.asarray(kernel, dtype=np.float32))
    X16 = ker.reshape(L2, P).astype(np.float16)
    Xf16 = X16.reshape(-1)

    times = []

    # ---- launch 1: inv1 = 1 / column sums of X (PE ones-matmul) -------
    in1 = []
    for m in range(NCORES):
        xp = np.zeros((L2P, PS), dtype=np.float16)
        xp[:L2] = X16[:, m * PS : (m + 1) * PS]
        in1.append({"xp": xp})
    r1 = _run("k1", _build_k1, in1, trace=trace)
    inv1 = np.concatenate([r["inv1"].ravel() for r in r1.results])
    times.append(r1.exec_time_ns)

    # ---- launch 2: per-chunk stage-2 normalize ------------------------
    # band m covers flat [NB*m, NB*(m+1)); element x there needs
    # inv1[(NB*m + x) % P]; NB % P == PS so the roll shift is PS*m.
    in2 = []
    for m in range(NCORES):
        i1b = np.resize(np.roll(inv1, -(PS * m) % P), NB).reshape(PS, L2)
        in2.append(
            {
                "a2": Xf16[NB * m : NB * (m + 1)].reshape(PS, L2),
                "i1b": np.ascontiguousarray(i1b).astype(np.float16),
            }
        )
    r2 = _run("k2", _build_k2, in2, trace=trace)
    B2 = np.concatenate([r["b2"] for r in r2.results], axis=0)  # (P, 361) f16
    times.append(r2.exec_time_ns)

    # ---- launch 3: final dot over B2T/U flat chunks (PE reduce) -------
    B2Tf = np.ascontiguousarray(B2.T).reshape(-1)
    pad = np.pad(inp[0, 2], L // 2, mode="reflect").astype(np.float16)
    from numpy.lib.stride_tricks import sliding_window_view

    U = np.ascontiguousarray(
        sliding_window_view(pad, (256, 256)).reshape(L2, P)
    )
    Uf = U.reshape(-1)
    in3 = []
    for m in range(NCORES):
        vT = np.zeros((L2P, PS), dtype=np.float16)
        uT = np.zeros((L2P, PS), dtype=np.float16)
        vT[:L2] = B2Tf[NB * m : NB * (m + 1)].reshape(PS, L2).T
        uT[:L2] = Uf[NB * m : NB * (m + 1)].reshape(PS, L2).T
        in3.append({"vT": vT, "uT": uT})
    r3 = _run("k3", _build_k3, in3, trace=trace)
    out = np.concatenate([r["o"].ravel() for r in r3.results])
    times.append(r3.exec_time_ns)

    if trace:
        kernel._last_times_ns = times  # stash for test harness

    return out.reshape(1, 1, 256, 256).astype(np.float32)


def hw_time_estimate_ns():
    """Per-launch HW time from the instruction cost model (TimelineSim).

    NTFF/neuron-profile capture is unavailable under this axon build, so this
    is the principled substitute: the same InstructionCostModel the Tile
    scheduler uses, over the exact BIR that runs on the cores.
    """
    from concourse.timeline_sim import TimelineSim

    out = []
    for key, builder in [("k1", _build_k1), ("k2", _build_k2), ("k3", _build_k3)]:
        if key not in _CACHE:
            _CACHE[key] = builder()
        out.append(int(TimelineSim(_CACHE[key]).simulate()))
    return out


# revision 10
# speedup vs baseline: 1.5333x; 1.0380x over previous
"""BatchBlur_SV kernel for 8 Trainium2 NeuronCores (Bass/Tile).

Reference semantics (including its reshape-scrambling "bug"):
  X = ker.reshape(361, 65536)                  # (kernel-pos ab, pixel p)
  s1 = X.sum(0);  W  = X / s1                  # stage-1 per-pixel normalize
  A2 = W.flat chunks of 361; s2 = row sums;  B2 = A2 / s2     # stage 2
  A3 = (B2.T).flat chunks of 361; s3 = row sums               # stage 3
  U  = im2col(reflect_pad(input[0,2], 9)) in (ab, p) layout   # (361, 65536)
  out[r] = sum(U.flat_chunk_r * A3[r]) / s3[r]

All arithmetic runs on-device in 3 SPMD launches over 8 cores. Host only
slices / rolls / transposes / dtype-converts between launches (data
movement, no math).

Big streams travel as fp16 (values are bounded and the gate is rel_err
< 2e-2); accumulations are fp32.  Column-direction reductions (k1 colsum,
k3 dots/s3) run on the PE array as ones-vector matmuls over a transposed
band layout, which is nearly free next to DVE reductions.
"""

import numpy as np

P = 65536          # pixels
L = 19
L2 = 361           # kernel positions
NCORES = 8
PS = P // NCORES   # 8192 rows per core
NB = PS * L2       # flat elements per band
G = 8              # subtiles per DMA group
NGRP = PS // (128 * G)   # 8 groups per core
NC = PS // 512     # 512-wide psum chunks per band (16)
CW = PS // 128     # per-partition width of [128, CW] strip relayout (64)

_CACHE: dict = {}


def _dt():
    from concourse import mybir
    return mybir.dt


def _grouped(ap):
    # (PS, L2) -> [g][k][(i j)] with row = g*1024 + k*G + i: each partition
    # holds G consecutive rows, so src/dst DMA patterns are contiguous 2D.
    return ap.rearrange("(g k i) j -> g k (i j)", g=NGRP, k=128, i=G)


_ROWS = [(0, 128), (128, 256), (256, L2)]  # partition tiles over the 361 rows
_BW = 2048                                 # column block width
_NBLK = PS // _BW                          # 4 blocks per band


def _build_k1():
    """colsum kernel: in xp (L2, PS) = X[:, pband] transposed slab;
    out inv1 (128, CW) with inv1[k, c] = 1 / s1[pband_start + k*CW + c].
    Column sums run on PE as ones-vector matmuls, pipelined in 2048-wide
    column blocks."""
    import concourse.bacc as bacc
    import concourse.tile as tile
    from concourse import mybir

    dt = _dt()
    nc = bacc.Bacc("TRN2", target_bir_lowering=False)
    xp = nc.dram_tensor("xp", [L2, PS], dt.float16, kind="ExternalInput")
    inv1 = nc.dram_tensor("inv1", [128, CW], dt.float32, kind="ExternalOutput")
    sc1 = nc.dram_tensor("sc1", [1, PS], dt.float32, kind="Internal")
    with tile.TileContext(nc) as tc:
        with (
            tc.tile_pool(name="io", bufs=3) as pool,
            tc.tile_pool(name="st", bufs=1) as spool,
            tc.psum_pool(name="ps", bufs=4) as psp,
        ):
            ones = spool.tile([128, 1], dt.float16)
            nc.vector.memset(ones, 1.0)
            strip = spool.tile([1, PS], dt.float32)
            for b in range(_NBLK):
                bsl = slice(_BW * b, _BW * (b + 1))
                xcs = []
                for t, (r0, r1) in enumerate(_ROWS):
                    xc = pool.tile([r1 - r0, _BW], dt.float16, tag=f"x{t}")
                    nc.sync.dma_start(out=xc, in_=xp[r0:r1, bsl])
                    xcs.append(xc)
                for s in range(_BW // 512):
                    c = (_BW // 512) * b + s
                    ps = psp.tile([1, 512], dt.float32)
                    for t, (r0, r1) in enumerate(_ROWS):
                        nc.tensor.matmul(
                            ps[:, :], lhsT=ones[: r1 - r0, :],
                            rhs=xcs[t][:, 512 * s : 512 * (s + 1)],
                            start=(t == 0), stop=(t == 2),
                        )
                    eng = nc.vector.tensor_copy if c % 2 else nc.scalar.copy
                    eng(out=strip[:, 512 * c : 512 * (c + 1)], in_=ps[:, :])
                nc.sync.dma_start(out=sc1[:, bsl], in_=strip[:, bsl])
            rt = spool.tile([128, CW], dt.float32)
            nc.sync.dma_start(
                out=rt, in_=sc1[:, :].rearrange("a (k c) -> (a k) c", k=128)
            )
            iv = spool.tile([128, CW], dt.float32)
            nc.vector.reciprocal(out=iv, in_=rt)
            nc.sync.dma_start(out=inv1[:, :], in_=iv)
    nc.compile()
    return nc


def _build_k2():
    """stage-2 kernel: in a2 (PS,361) = X.flat band, i1b (PS,361) = matching
    per-element stage-1 reciprocal sums; out b2 (PS,361) normalized chunks."""
    import concourse.bacc as bacc
    import concourse.tile as tile
    from concourse import mybir

    dt = _dt()
    nc = bacc.Bacc("TRN2", target_bir_lowering=False)
    a2 = nc.dram_tensor("a2", [PS, L2], dt.float16, kind="ExternalInput")
    i1b = nc.dram_tensor("i1b", [PS, L2], dt.float16, kind="ExternalInput")
    b2 = nc.dram_tensor("b2", [PS, L2], dt.float16, kind="ExternalOutput")
    a2r, i1r, b2r = _grouped(a2[:, :]), _grouped(i1b[:, :]), _grouped(b2[:, :])
    with tile.TileContext(nc) as tc:
        with (
            tc.tile_pool(name="io", bufs=3) as pool,
            tc.tile_pool(name="w", bufs=2) as wpool,
            tc.tile_pool(name="st", bufs=3) as spool,
        ):
            for g in range(NGRP):
                ta = pool.tile([128, G, L2], dt.float16, tag="ta")
                ti = pool.tile([128, G, L2], dt.float16, tag="ti")
                nc.sync.dma_start(
                    out=ta[:, :, :].rearrange("k i j -> k (i j)"), in_=a2r[g]
                )
                nc.sync.dma_start(
                    out=ti[:, :, :].rearrange("k i j -> k (i j)"), in_=i1r[g]
                )
                tw = wpool.tile([128, G, L2], dt.float16, tag="tw")
                nc.vector.tensor_mul(out=tw, in0=ta, in1=ti)
                s2 = spool.tile([128, G], dt.float32, tag="s2")
                nc.vector.tensor_reduce(
                    out=s2, in_=tw,
                    axis=mybir.AxisListType.X, op=mybir.AluOpType.add,
                )
                r2 = spool.tile([128, G], dt.float32, tag="r2")
                nc.vector.reciprocal(out=r2, in_=s2)
                tb = wpool.tile([128, G, L2], dt.float16, tag="tb")
                for i in range(G):
                    nc.vector.tensor_scalar_mul(
                        out=tb[:, i, :], in0=tw[:, i, :],
                        scalar1=r2[:, i : i + 1],
                    )
                nc.sync.dma_start(
                    out=b2r[g], in_=tb[:, :, :].rearrange("k i j -> k (i j)")
                )
    nc.compile()
    return nc


def _build_k3():
    """final kernel: in vT/uT (L2, PS) = transposed B2T/U flat bands;
    out o (128, CW) with o[k, c] = out[band_start + k*CW + c]."""
    import concourse.bacc as bacc
    import concourse.tile as tile
    from concourse import mybir

    dt = _dt()
    nc = bacc.Bacc("TRN2", target_bir_lowering=False)
    vT = nc.dram_tensor("vT", [L2, PS], dt.float16, kind="ExternalInput")
    uT = nc.dram_tensor("uT", [L2, PS], dt.float16, kind="ExternalInput")
    o = nc.dram_tensor("o", [128, CW], dt.float32, kind="ExternalOutput")
    scd = nc.dram_tensor("scd", [1, PS], dt.float16, kind="Internal")
    scs = nc.dram_tensor("scs", [1, PS], dt.float16, kind="Internal")
    with tile.TileContext(nc) as tc:
        with (
            tc.tile_pool(name="io", bufs=3) as pool,
            tc.tile_pool(name="pr", bufs=3) as prp,
            tc.tile_pool(name="st", bufs=1) as spool,
            tc.psum_pool(name="ps", bufs=4) as psp,
        ):
            ones = spool.tile([128, 1], dt.float16)
            nc.vector.memset(ones, 1.0)
            sdot = spool.tile([1, PS], dt.float16, tag="sdot")
            ssum = spool.tile([1, PS], dt.float16, tag="ssum")
            for b in range(_NBLK):
                bsl = slice(_BW * b, _BW * (b + 1))
                vcs, prods = [], []
                for t, (r0, r1) in enumerate(_ROWS):
                    vc = pool.tile([r1 - r0, _BW], dt.float16, tag=f"v{t}")
                    uc = pool.tile([r1 - r0, _BW], dt.float16, tag=f"u{t}")
                    nc.sync.dma_start(out=vc, in_=vT[r0:r1, bsl])
                    nc.sync.dma_start(out=uc, in_=uT[r0:r1, bsl])
                    pr = prp.tile([r1 - r0, _BW], dt.float16, tag=f"p{t}")
                    nc.vector.tensor_mul(out=pr, in0=vc, in1=uc)
                    vcs.append(vc)
                    prods.append(pr)
                for s in range(_BW // 512):
                    sl = slice(512 * s, 512 * (s + 1))
                    gsl = slice(_BW * b + 512 * s, _BW * b + 512 * (s + 1))
                    psd = psp.tile([1, 512], dt.float32, tag="psd")
                    for t, (r0, r1) in enumerate(_ROWS):
                        nc.tensor.matmul(
                            psd[:, :], lhsT=ones[: r1 - r0, :],
                            rhs=prods[t][:, sl],
                            start=(t == 0), stop=(t == 2),
                        )
                    pss = psp.tile([1, 512], dt.float32, tag="pss")
                    for t, (r0, r1) in enumerate(_ROWS):
                        nc.tensor.matmul(
                            pss[:, :], lhsT=ones[: r1 - r0, :],
                            rhs=vcs[t][:, sl],
                            start=(t == 0), stop=(t == 2),
                        )
                    nc.vector.tensor_copy(out=sdot[:, gsl], in_=psd[:, :])
                    nc.scalar.copy(out=ssum[:, gsl], in_=pss[:, :])
                nc.sync.dma_start(out=scd[:, bsl], in_=sdot[:, bsl])
                nc.sync.dma_start(out=scs[:, bsl], in_=ssum[:, bsl])
            td = spool.tile([128, CW], dt.float16, tag="td")
            ts = spool.tile([128, CW], dt.float16, tag="ts")
            nc.sync.dma_start(out=td, in_=scd[:, :].rearrange("a (k c) -> (a k) c", k=128))
            nc.sync.dma_start(out=ts, in_=scs[:, :].rearrange("a (k c) -> (a k) c", k=128))
            tr = spool.tile([128, CW], dt.float32, tag="tr")
            nc.vector.reciprocal(out=tr, in_=ts)
            to = spool.tile([128, CW], dt.float32, tag="to")
            nc.vector.tensor_mul(out=to, in0=td, in1=tr)
            nc.sync.dma_start(out=o[:, :], in_=to)
    nc.compile()
    return nc


def _run(key, builder, in_maps, trace=False):
    from concourse.bass_utils import run_bass_kernel_spmd

    if key not in _CACHE:
        _CACHE[key] = builder()
    res = run_bass_kernel_spmd(
        _CACHE[key], in_maps, core_ids=list(range(NCORES)), trace=trace
    )
    return res


def kernel(input, kernel):
    import os

    trace = bool(int(os.environ.get("BASSBLUR_TRACE", "0")))
    inp = np.ascontiguousarray(np.asarray(input, dtype=np.float32))
    ker = np.ascontiguousarray(np.asarray(kernel, dtype=np.float32))
    X16 = ker.reshape(L2, P).astype(np.float16)
    Xf16 = X16.reshape(-1)

    times = []

    # ---- launch 1: inv1 = 1 / column sums of X (PE ones-matmul) -------
    in1 = [
        {"xp": np.ascontiguousarray(X16[:, m * PS : (m + 1) * PS])}
        for m in range(NCORES)
    ]
    r1 = _run("k1", _build_k1, in1, trace=trace)
    inv1 = np.concatenate([r["inv1"].ravel() for r in r1.results])
    times.append(r1.exec_time_ns)

    # ---- launch 2: per-chunk stage-2 normalize ------------------------
    # band m covers flat [NB*m, NB*(m+1)); element x there needs
    # inv1[(NB*m + x) % P]; NB % P == PS so the roll shift is PS*m.
    in2 = []
    for m in range(NCORES):
        i1b = np.resize(np.roll(inv1, -(PS * m) % P), NB).reshape(PS, L2)
        in2.append(
            {
                "a2": Xf16[NB * m : NB * (m + 1)].reshape(PS, L2),
                "i1b": np.ascontiguousarray(i1b).astype(np.float16),
            }
        )
    r2 = _run("k2", _build_k2, in2, trace=trace)
    B2 = np.concatenate([r["b2"] for r in r2.results], axis=0)  # (P, 361) f16
    times.append(r2.exec_time_ns)

    # ---- launch 3: final dot over B2T/U flat chunks (PE reduce) -------
    B2Tf = np.ascontiguousarray(B2.T).reshape(-1)
    pad = np.pad(inp[0, 2], L // 2, mode="reflect").astype(np.float16)
    from numpy.lib.stride_tricks import sliding_window_view

    U = np.ascontiguousarray(
        sliding_window_view(pad, (256, 256)).reshape(L2, P)
    )
    Uf = U.reshape(-1)
    in3 = [
        {
            "vT": np.ascontiguousarray(
                B2Tf[NB * m : NB * (m + 1)].reshape(PS, L2).T
            ),
            "uT": np.ascontiguousarray(
                Uf[NB * m : NB * (m + 1)].reshape(PS, L2).T
            ),
        }
        for m in range(NCORES)
    ]
    r3 = _run("k3", _build_k3, in3, trace=trace)
    out = np.concatenate([r["o"].ravel() for r in r3.results])
    times.append(r3.exec_time_ns)

    if trace:
        kernel._last_times_ns = times  # stash for test harness

    return out.reshape(1, 1, 256, 256).astype(np.float32)


def hw_time_estimate_ns():
    """Per-launch HW time from the instruction cost model (TimelineSim).

    NTFF/neuron-profile capture is unavailable under this axon build, so this
    is the principled substitute: the same InstructionCostModel the Tile
    scheduler uses, over the exact BIR that runs on the cores.
    """
    from concourse.timeline_sim import TimelineSim

    out = []
    for key, builder in [("k1", _build_k1), ("k2", _build_k2), ("k3", _build_k3)]:
        if key not in _CACHE:
            _CACHE[key] = builder()
        out.append(int(TimelineSim(_CACHE[key]).simulate()))
    return out


# revision 12
# speedup vs baseline: 1.5779x; 1.0291x over previous
"""BatchBlur_SV kernel for 8 Trainium2 NeuronCores (Bass/Tile).

Reference semantics (including its reshape-scrambling "bug"):
  X = ker.reshape(361, 65536)                  # (kernel-pos ab, pixel p)
  s1 = X.sum(0);  W  = X / s1                  # stage-1 per-pixel normalize
  A2 = W.flat chunks of 361; s2 = row sums;  B2 = A2 / s2     # stage 2
  A3 = (B2.T).flat chunks of 361; s3 = row sums               # stage 3
  U  = im2col(reflect_pad(input[0,2], 9)) in (ab, p) layout   # (361, 65536)
  out[r] = sum(U.flat_chunk_r * A3[r]) / s3[r]

All arithmetic runs on-device in 3 SPMD launches over 8 cores. Host only
slices / rolls / transposes / dtype-converts between launches (data
movement, no math).

Big streams travel as fp16 (values are bounded and the gate is rel_err
< 2e-2); accumulations are fp32.  Column-direction reductions (k1 colsum,
k3 dots/s3) run on the PE array as ones-vector matmuls over a transposed
band layout, which is nearly free next to DVE reductions.
"""

import numpy as np

P = 65536          # pixels
L = 19
L2 = 361           # kernel positions
NCORES = 8
PS = P // NCORES   # 8192 rows per core
NB = PS * L2       # flat elements per band
G = 8              # subtiles per DMA group
NGRP = PS // (128 * G)   # 8 groups per core
NC = PS // 512     # 512-wide psum chunks per band (16)
CW = PS // 128     # per-partition width of [128, CW] strip relayout (64)

_CACHE: dict = {}


def _dt():
    from concourse import mybir
    return mybir.dt


def _grouped(ap):
    # (PS, L2) -> [g][k][(i j)] with row = g*1024 + k*G + i: each partition
    # holds G consecutive rows, so src/dst DMA patterns are contiguous 2D.
    return ap.rearrange("(g k i) j -> g k (i j)", g=NGRP, k=128, i=G)


_ROWS = [(0, 128), (128, 256), (256, L2)]  # partition tiles over the 361 rows
_BW = 2048                                 # column block width
_NBLK = PS // _BW                          # 4 blocks per band


def _build_k1():
    """colsum kernel: in xp (L2, PS) = X[:, pband] transposed slab;
    out inv1 (1, PS) strip with inv1[0, i] = 1 / s1[pband_start + i].
    Column sums run on PE as ones-vector matmuls, pipelined in 2048-wide
    column blocks; reciprocals run on the Pool engine per 512-chunk."""
    import concourse.bacc as bacc
    import concourse.tile as tile
    from concourse import mybir

    dt = _dt()
    nc = bacc.Bacc("TRN2", target_bir_lowering=False)
    xp = nc.dram_tensor("xp", [L2, PS], dt.float16, kind="ExternalInput")
    inv1 = nc.dram_tensor("inv1", [1, PS], dt.float32, kind="ExternalOutput")
    with tile.TileContext(nc) as tc:
        with (
            tc.tile_pool(name="io", bufs=3) as pool,
            tc.tile_pool(name="st", bufs=1) as spool,
            tc.psum_pool(name="ps", bufs=4) as psp,
        ):
            ones = spool.tile([128, 1], dt.float16)
            nc.vector.memset(ones, 1.0)
            strip = spool.tile([1, PS], dt.float16)
            istrip = spool.tile([1, PS], dt.float32)
            for b in range(_NBLK):
                bsl = slice(_BW * b, _BW * (b + 1))
                xcs = []
                for t, (r0, r1) in enumerate(_ROWS):
                    xc = pool.tile([r1 - r0, _BW], dt.float16, tag=f"x{t}")
                    nc.sync.dma_start(out=xc, in_=xp[r0:r1, bsl])
                    xcs.append(xc)
                for s in range(_BW // 512):
                    c = (_BW // 512) * b + s
                    csl = slice(512 * c, 512 * (c + 1))
                    ps = psp.tile([1, 512], dt.float32)
                    for t, (r0, r1) in enumerate(_ROWS):
                        nc.tensor.matmul(
                            ps[:, :], lhsT=ones[: r1 - r0, :],
                            rhs=xcs[t][:, 512 * s : 512 * (s + 1)],
                            start=(t == 0), stop=(t == 2),
                        )
                    eng = nc.vector.tensor_copy if c % 2 else nc.scalar.copy
                    eng(out=strip[:, csl], in_=ps[:, :])
                    nc.vector.reciprocal(out=istrip[:, csl], in_=strip[:, csl])
                nc.sync.dma_start(out=inv1[:, bsl], in_=istrip[:, bsl])
    nc.compile()
    return nc


def _build_k2():
    """stage-2 kernel: in a2 (PS,361) = X.flat band, i1b (PS,361) = matching
    per-element stage-1 reciprocal sums; out b2 (PS,361) normalized chunks."""
    import concourse.bacc as bacc
    import concourse.tile as tile
    from concourse import mybir

    dt = _dt()
    nc = bacc.Bacc("TRN2", target_bir_lowering=False)
    a2 = nc.dram_tensor("a2", [PS, L2], dt.float16, kind="ExternalInput")
    i1b = nc.dram_tensor("i1b", [PS, L2], dt.float16, kind="ExternalInput")
    b2 = nc.dram_tensor("b2", [PS, L2], dt.float16, kind="ExternalOutput")
    a2r, i1r, b2r = _grouped(a2[:, :]), _grouped(i1b[:, :]), _grouped(b2[:, :])
    with tile.TileContext(nc) as tc:
        with (
            tc.tile_pool(name="io", bufs=3) as pool,
            tc.tile_pool(name="w", bufs=2) as wpool,
            tc.tile_pool(name="st", bufs=3) as spool,
        ):
            for g in range(NGRP):
                ta = pool.tile([128, G, L2], dt.float16, tag="ta")
                ti = pool.tile([128, G, L2], dt.float16, tag="ti")
                nc.sync.dma_start(
                    out=ta[:, :, :].rearrange("k i j -> k (i j)"), in_=a2r[g]
                )
                nc.sync.dma_start(
                    out=ti[:, :, :].rearrange("k i j -> k (i j)"), in_=i1r[g]
                )
                tw = wpool.tile([128, G, L2], dt.float16, tag="tw")
                nc.vector.tensor_mul(out=tw, in0=ta, in1=ti)
                s2 = spool.tile([128, G], dt.float32, tag="s2")
                nc.vector.tensor_reduce(
                    out=s2, in_=tw,
                    axis=mybir.AxisListType.X, op=mybir.AluOpType.add,
                )
                r2 = spool.tile([128, G], dt.float32, tag="r2")
                nc.vector.reciprocal(out=r2, in_=s2)
                tb = wpool.tile([128, G, L2], dt.float16, tag="tb")
                for i in range(G):
                    nc.vector.tensor_scalar_mul(
                        out=tb[:, i, :], in0=tw[:, i, :],
                        scalar1=r2[:, i : i + 1],
                    )
                nc.sync.dma_start(
                    out=b2r[g], in_=tb[:, :, :].rearrange("k i j -> k (i j)")
                )
    nc.compile()
    return nc


def _build_k3():
    """final kernel: in vT/uT (L2, PS) = transposed B2T/U flat bands;
    out o (1, PS) strip with o[0, i] = out[band_start + i]."""
    import concourse.bacc as bacc
    import concourse.tile as tile
    from concourse import mybir

    dt = _dt()
    nc = bacc.Bacc("TRN2", target_bir_lowering=False)
    vT = nc.dram_tensor("vT", [L2, PS], dt.float16, kind="ExternalInput")
    uT = nc.dram_tensor("uT", [L2, PS], dt.float16, kind="ExternalInput")
    o = nc.dram_tensor("o", [1, PS], dt.float32, kind="ExternalOutput")
    with tile.TileContext(nc) as tc:
        with (
            tc.tile_pool(name="io", bufs=3) as pool,
            tc.tile_pool(name="pr", bufs=3) as prp,
            tc.tile_pool(name="st", bufs=1) as spool,
            tc.psum_pool(name="ps", bufs=4) as psp,
        ):
            ones = spool.tile([128, 1], dt.float16)
            nc.vector.memset(ones, 1.0)
            sdot = spool.tile([1, PS], dt.float16, tag="sdot")
            ssum = spool.tile([1, PS], dt.float16, tag="ssum")
            rstr = spool.tile([1, PS], dt.float32, tag="rstr")
            ostr = spool.tile([1, PS], dt.float32, tag="ostr")
            for b in range(_NBLK):
                bsl = slice(_BW * b, _BW * (b + 1))
                vcs, prods = [], []
                for t, (r0, r1) in enumerate(_ROWS):
                    vc = pool.tile([r1 - r0, _BW], dt.float16, tag=f"v{t}")
                    uc = pool.tile([r1 - r0, _BW], dt.float16, tag=f"u{t}")
                    nc.sync.dma_start(out=vc, in_=vT[r0:r1, bsl])
                    nc.sync.dma_start(out=uc, in_=uT[r0:r1, bsl])
                    pr = prp.tile([r1 - r0, _BW], dt.float16, tag=f"p{t}")
                    nc.vector.tensor_mul(out=pr, in0=vc, in1=uc)
                    vcs.append(vc)
                    prods.append(pr)
                for s in range(_BW // 512):
                    sl = slice(512 * s, 512 * (s + 1))
                    gsl = slice(_BW * b + 512 * s, _BW * b + 512 * (s + 1))
                    psd = psp.tile([1, 512], dt.float32, tag="psd")
                    for t, (r0, r1) in enumerate(_ROWS):
                        nc.tensor.matmul(
                            psd[:, :], lhsT=ones[: r1 - r0, :],
                            rhs=prods[t][:, sl],
                            start=(t == 0), stop=(t == 2),
                        )
                    pss = psp.tile([1, 512], dt.float32, tag="pss")
                    for t, (r0, r1) in enumerate(_ROWS):
                        nc.tensor.matmul(
                            pss[:, :], lhsT=ones[: r1 - r0, :],
                            rhs=vcs[t][:, sl],
                            start=(t == 0), stop=(t == 2),
                        )
                    nc.scalar.copy(out=sdot[:, gsl], in_=psd[:, :])
                    nc.scalar.copy(out=ssum[:, gsl], in_=pss[:, :])
                    nc.vector.reciprocal(out=rstr[:, gsl], in_=ssum[:, gsl])
                nc.vector.tensor_mul(
                    out=ostr[:, bsl], in0=sdot[:, bsl], in1=rstr[:, bsl]
                )
                nc.sync.dma_start(out=o[:, bsl], in_=ostr[:, bsl])
    nc.compile()
    return nc


def _run(key, builder, in_maps, trace=False):
    from concourse.bass_utils import run_bass_kernel_spmd

    if key not in _CACHE:
        _CACHE[key] = builder()
    res = run_bass_kernel_spmd(
        _CACHE[key], in_maps, core_ids=list(range(NCORES)), trace=trace
    )
    return res


def kernel(input, kernel):
    import os

    trace = bool(int(os.environ.get("BASSBLUR_TRACE", "0")))
    inp = np.ascontiguousarray(np.asarray(input, dtype=np.float32))
    ker = np.ascontiguousarray(np.asarray(kernel, dtype=np.float32))
    X16 = ker.reshape(L2, P).astype(np.float16)
    Xf16 = X16.reshape(-1)

    times = []

    # ---- launch 1: inv1 = 1 / column sums of X (PE ones-matmul) -------
    in1 = [
        {"xp": np.ascontiguousarray(X16[:, m * PS : (m + 1) * PS])}
        for m in range(NCORES)
    ]
    r1 = _run("k1", _build_k1, in1, trace=trace)
    inv1 = np.concatenate([r["inv1"][0] for r in r1.results])
    times.append(r1.exec_time_ns)

    # ---- launch 2: per-chunk stage-2 normalize ------------------------
    # band m covers flat [NB*m, NB*(m+1)); element x there needs
    # inv1[(NB*m + x) % P]; NB % P == PS so the roll shift is PS*m.
    in2 = []
    for m in range(NCORES):
        i1b = np.resize(np.roll(inv1, -(PS * m) % P), NB).reshape(PS, L2)
        in2.append(
            {
                "a2": Xf16[NB * m : NB * (m + 1)].reshape(PS, L2),
                "i1b": np.ascontiguousarray(i1b).astype(np.float16),
            }
        )
    r2 = _run("k2", _build_k2, in2, trace=trace)
    B2 = np.concatenate([r["b2"] for r in r2.results], axis=0)  # (P, 361) f16
    times.append(r2.exec_time_ns)

    # ---- launch 3: final dot over B2T/U flat chunks (PE reduce) -------
    B2Tf = np.ascontiguousarray(B2.T).reshape(-1)
    pad = np.pad(inp[0, 2], L // 2, mode="reflect").astype(np.float16)
    from numpy.lib.stride_tricks import sliding_window_view

    U = np.ascontiguousarray(
        sliding_window_view(pad, (256, 256)).reshape(L2, P)
    )
    Uf = U.reshape(-1)
    in3 = [
        {
            "vT": np.ascontiguousarray(
                B2Tf[NB * m : NB * (m + 1)].reshape(PS, L2).T
            ),
            "uT": np.ascontiguousarray(
                Uf[NB * m : NB * (m + 1)].reshape(PS, L2).T
            ),
        }
        for m in range(NCORES)
    ]
    r3 = _run("k3", _build_k3, in3, trace=trace)
    out = np.concatenate([r["o"][0] for r in r3.results])
    times.append(r3.exec_time_ns)

    if trace:
        kernel._last_times_ns = times  # stash for test harness

    return out.reshape(1, 1, 256, 256).astype(np.float32)


def hw_time_estimate_ns():
    """Per-launch HW time from the instruction cost model (TimelineSim).

    NTFF/neuron-profile capture is unavailable under this axon build, so this
    is the principled substitute: the same InstructionCostModel the Tile
    scheduler uses, over the exact BIR that runs on the cores.
    """
    from concourse.timeline_sim import TimelineSim

    out = []
    for key, builder in [("k1", _build_k1), ("k2", _build_k2), ("k3", _build_k3)]:
        if key not in _CACHE:
            _CACHE[key] = builder()
        out.append(int(TimelineSim(_CACHE[key]).simulate()))
    return out
